# revision 48
# baseline (speedup 1.0000x reference)
"""Trainium2 Bass kernel for nn_MultiHeadAttention_67731634258682.

MHA: B=2, S=8192, D=1024, H=16 heads (depth 64).
Sharding over 8 cores: core c -> (batch b = c//4, head-group g = c%4).
Each core computes its 4 heads end-to-end plus a row-parallel partial of
the output projection; the host sums the 4 partials per batch.

Design (measured 1.88 ms; the earlier session's kernel was 2.06 ms and
the staged nn baseline 3.09 ms on this box):
  - Everything SBUF-resident: K^T/Q^T (2 pairs x [128, S]), V' (2 pairs x
    [128, nkc, 130] with ones columns), pair-0 O^T [128, S]. Projections
    evacuate straight into the resident tiles (no DRAM scratch round-trip).
  - Exp split 1:1 across engines by k-chunk parity: ScalarE exact Exp
    (softmax scale folded into the activation) / DVE Schraudolph
    bit-trick: bf16 bits = int16(round(A*logit + B)), one fused
    tensor_scalar per slot. Rel-err measured 1.42e-2 (gate 2e-2).
  - k-loop processes BLOCKS of 2 k-chunks with a 3-deep logits PSUM ring
    (3 x 2 banks, shared by all small PE outputs) + 2 pinned PV banks.
    Per block: pv(j) x4 (P tiles made a block ago), lg(j+4) x2 row-tiled
    pairs (tile_position (0,0)/(64,0) -> the two heads' QK^T matmuls
    stream CONCURRENTLY), exp(j+2) x2 emitted a block early so they sit
    ahead of drained-closure ops in the ACT/DVE queues; lg and exp
    lookahead both run across tile boundaries so the PE never drains at
    a tile turn. pv-before-lg leaves every matmul one pending LDWEIGHTS.
  - Epilogue per tile (no PE transposes): U'=[O_unnorm^T; denom] spills
    to SBUF (one head on ACT, one on DVE), then deferred closures:
    denom row staged to partition 0 on GpSimd, reciprocal_approx_fast on
    DVE (plain reciprocal is ~8 cyc/elem; approx NaNs on nonzero base
    partitions, hence the staging), bf16 cast on GpSimd, K=1 ones-row
    matmul broadcasts the recip row across partitions, one fused DVE
    tensor_tensor multiply writes the resident O^T. Wo chunks are one
    (qs, f) each so a drain block carries at most one ysb evacuation.
  - V-projection carries no bias: sum p*(v+bv) = sum p*v + bv*denom,
    i.e. +bv after normalization, i.e. +bv@Wo on Y -- a constant row
    added on the host (exact, in float64).
  - Only the first few s-chunks of the K/V projections run up front
    (split across two DMA queues; ~145 GB/s per queue on this strided
    pattern); the rest drain as deadline-ordered closures inside the
    first attention tile. Trailing weights (Wv/Wq/Wo/bq) issue after
    the first k-chunk loads so they never delay the first matmul.
"""

import os
import sys
import numpy as np

for _p in ("/opt/trn_rl_repo", "/root/.axon_site/_ro/trn_rl_repo"):
    if os.path.isdir(_p) and _p not in sys.path:
        sys.path.append(_p)

import concourse.bass as bass
import concourse.mybir as mybir
from concourse import bacc, tile
from concourse.bass import ts, ds
from concourse.bass_utils import run_bass_kernel_spmd

F32 = mybir.dt.float32
BF16 = mybir.dt.bfloat16
I16 = mybir.dt.int16

B, S, D = 2, 8192, 1024
H = 16
DEPTH = 64          # head dim
G = 4               # head groups (one per core within a batch)
HPG = 4             # heads per group
DG = HPG * DEPTH    # 256 features per group
QT = 512            # q tile
KC = 128            # k chunk (matmul contraction tile)
NDC = D // 128      # 8 contraction chunks for projections

AFT = mybir.ActivationFunctionType
ALU = mybir.AluOpType

SCALE = 0.125                                  # 1/sqrt(64)
SCH_A = SCALE * np.log2(np.e) * 128.0          # schraudolph multiplier
SCH_B0 = 127.0 * 128.0                         # exponent bias in bf16 bits


def build_program(seq=S, dve_num=1, dve_den=3, boff=-7.4):
    """Build the per-core Bass program. Returns the compiled Bacc object."""
    assert seq % QT == 0
    nqt = seq // QT
    nkc = seq // KC
    nsc = seq // QT
    dt = BF16

    nc = bacc.Bacc("TRN2", target_bir_lowering=False, debug=False,
                   enable_asserts=False, num_devices=8)

    # ---- external I/O ----
    qT = nc.dram_tensor("qT", [D, seq], dt, kind="ExternalInput").ap()
    kT = nc.dram_tensor("kT", [D, seq], dt, kind="ExternalInput").ap()
    vT = nc.dram_tensor("vT", [D, seq], dt, kind="ExternalInput").ap()
    Wq = nc.dram_tensor("Wq", [D, DG], dt, kind="ExternalInput").ap()
    Wk = nc.dram_tensor("Wk", [D, DG], dt, kind="ExternalInput").ap()
    Wv = nc.dram_tensor("Wv", [D, DG], dt, kind="ExternalInput").ap()
    Wo = nc.dram_tensor("Wo", [DG, D], dt, kind="ExternalInput").ap()
    bq = nc.dram_tensor("bq", [DG, 1], F32, kind="ExternalInput").ap()
    bk = nc.dram_tensor("bk", [DG, 1], F32, kind="ExternalInput").ap()
    Y = nc.dram_tensor("Y", [seq, D], F32, kind="ExternalOutput").ap()

    with tile.TileContext(nc) as tc:
        from contextlib import ExitStack
        ctx = ExitStack()
        with ctx:
            const = ctx.enter_context(tc.tile_pool(name="const", bufs=1))
            res = ctx.enter_context(tc.tile_pool(name="res", bufs=1))
            xin = ctx.enter_context(tc.tile_pool(name="xin", bufs=3))
            ppool = ctx.enter_context(tc.tile_pool(name="ppool", bufs=3))
            epi = ctx.enter_context(tc.tile_pool(name="epi", bufs=4))
            otp = ctx.enter_context(tc.tile_pool(name="otp", bufs=3))
            ypool = ctx.enter_context(tc.tile_pool(name="ypool", bufs=3))
            # One shared PSUM ring: 3 slots of 2 banks each (tag "lg") serve
            # the logits tiles AND all small PE outputs (transposes, Wo
            # accumulator, projection accumulator); pv pins the last 2 banks.
            ps_logit = ctx.enter_context(
                tc.tile_pool(name="ps_logit", bufs=3, space="PSUM"))
            ps_pv = ctx.enter_context(
                tc.tile_pool(name="ps_pv", bufs=1, space="PSUM"))

            def ps_tile(shape, name):
                return ps_logit.tile(shape, F32, tag="lg", bufs=3, name=name)

            # ---- constants ----
            ones_f32 = const.tile([128, 128], F32, tag="ones_f32")
            nc.any.memset(ones_f32[:], 1.0)
            ones_row = const.tile([1, 128], dt, tag="ones_row")
            nc.vector.tensor_copy(ones_row[:], ones_f32[0:1, :])

            # Startup DMA order matters: the first k-proj matmul needs wk +
            # bk + the first kT chunk. Spread issues across engine queues
            # (issue cost ~0.6us each on one queue) and front-load the
            # K-path; wq/wo can trail (first used much later).
            wq_sb = [const.tile([128, DG], dt, tag=f"wq{dc}", name=f"wq{dc}")
                     for dc in range(NDC)]
            wk_sb = [const.tile([128, DG], dt, tag=f"wk{dc}", name=f"wk{dc}")
                     for dc in range(NDC)]
            wv_sb = [const.tile([128, DG], dt, tag=f"wv{dc}", name=f"wv{dc}")
                     for dc in range(NDC)]
            qs_engines = [nc.scalar, nc.gpsimd]
            bk_sb = [const.tile([128, 1], F32, tag=f"bk{i}", name=f"bk{i}")
                     for i in range(2)]
            bq_sb = [const.tile([128, 1], F32, tag=f"bq{i}", name=f"bq{i}")
                     for i in range(2)]
            for dc in range(NDC):
                qs_engines[dc % 2].dma_start(wk_sb[dc][:], Wk[ts(dc, 128), :])
            for i in range(2):
                qs_engines[i].dma_start(bk_sb[i][:], bk[ts(i, 128), :])
            wo_sb = [const.tile([128, D], dt, tag=f"wo{i}", name=f"wo{i}")
                     for i in range(2)]

            def load_late_weights():
                # issued after the PRE_K x loads so the scalar DMA queue
                # serves the first k-chunks before these trailing weights.
                for dc in range(NDC):
                    qs_engines[dc % 2].dma_start(
                        wv_sb[dc][:], Wv[ts(dc, 128), :])
                for dc in range(NDC):
                    qs_engines[dc % 2].dma_start(
                        wq_sb[dc][:], Wq[ts(dc, 128), :])
                for i in range(2):
                    qs_engines[i].dma_start(bq_sb[i][:], bq[ts(i, 128), :])
                for i in range(2):
                    qs_engines[i].dma_start(wo_sb[i][:], Wo[ts(i, 128), :])

            # ---- resident tensors (persist for the whole kernel) ----
            ktr = [res.tile([128, seq], dt, tag=f"ktr{p}", name=f"ktr{p}")
                   for p in range(2)]
            qtr = [res.tile([128, seq], dt, tag=f"qtr{p}", name=f"qtr{p}")
                   for p in range(2)]
            VP_W = 2 * (DEPTH + 1)  # per-pair per-chunk: 2 heads x [V_h|1]
            vpr = [res.tile([128, nkc, VP_W], dt, tag=f"vpr{p}",
                            name=f"vpr{p}") for p in range(2)]
            ot0 = res.tile([128, seq], dt, tag="ot0", name="ot0")
            # ones columns of V' are constant: write once.
            for p in range(2):
                vh = vpr[p][:].rearrange("p k (h x) -> p k h x", x=DEPTH + 1)
                nc.vector.memset(vh[:, :, :, DEPTH:DEPTH + 1], 1.0)

            # ---- projection emitters ----
            def load_x(src, sc, eng, eng2=None, eng3=None):
                # eng2/eng3 split the 1MB strided load across DMA queues
                # (one engine sustains only ~145GB/s on this pattern) --
                # used for the up-front P1 loads that gate the first tile.
                xt = xin.tile([128, NDC, QT], dt, tag="xt", name="xt")
                rr = src.rearrange("(c p) s -> p c s", p=128)
                if eng2 is None:
                    eng.dma_start(xt[:], rr[:, :, ts(sc, QT)])
                elif eng3 is None:
                    hc = NDC // 2
                    eng.dma_start(xt[:, 0:hc, :],
                                  rr[:, 0:hc, ts(sc, QT)])
                    eng2.dma_start(xt[:, hc:NDC, :],
                                   rr[:, hc:NDC, ts(sc, QT)])
                else:
                    tc_ = NDC // 3 + 1
                    eng.dma_start(xt[:, 0:tc_, :],
                                  rr[:, 0:tc_, ts(sc, QT)])
                    eng2.dma_start(xt[:, tc_:2 * tc_, :],
                                   rr[:, tc_:2 * tc_, ts(sc, QT)])
                    eng3.dma_start(xt[:, 2 * tc_:NDC, :],
                                   rr[:, 2 * tc_:NDC, ts(sc, QT)])
                return xt

            def qkproj_emit(sc, xt, w_sb, b_sb, dst, f, on_act=False):
                ps = ps_tile([128, QT], "ps")
                for dc in range(NDC):
                    nc.tensor.matmul(
                        ps[:], w_sb[dc][:, ts(f, 128)], xt[:, dc, :],
                        start=(dc == 0), stop=(dc == NDC - 1))
                if on_act:
                    # P1 runs before attention: ScalarE is idle there, so
                    # evacuate on it to keep DVE free.
                    nc.scalar.activation(
                        dst[f][:, ts(sc, QT)], ps[:], AFT.Identity,
                        bias=b_sb[f][:])
                else:
                    nc.vector.tensor_scalar_add(
                        dst[f][:, ts(sc, QT)], ps[:], b_sb[f][:])

            def vproj_emit(sc, xt, sub):
                # no bias ride-along: out = sum p*(v+bv) differs from
                # sum p*v by bv*denom, i.e. +bv after normalization, i.e.
                # +bv@Wo on Y -- a constant row the host adds exactly.
                ps = ps_tile([128, DG], "ps")
                for dc in range(NDC):
                    nc.tensor.matmul(
                        ps[:], xt[:, dc, ts(sub, 128)], wv_sb[dc][:],
                        start=(dc == 0), stop=(dc == NDC - 1))
                kc_i = sc * (QT // 128) + sub
                for p in range(2):
                    src = ps[:, ds(p * 2 * DEPTH, 2 * DEPTH)].rearrange(
                        "p (h x) -> p h x", x=DEPTH)
                    dstv = vpr[p][:, kc_i, :].rearrange(
                        "p (h x) -> p h x", x=DEPTH + 1)
                    nc.scalar.copy(dstv[:, :, 0:DEPTH], src)

            # ================= P1: K + V projections =================
            # Only the first few s-chunks are projected up front (the ones
            # the first attention tile touches before its deferred chunks
            # can land); the rest are deadline-ordered closures drained
            # into tile (0,0)'s k-loop, filling its coupling idle.
            PRE_K = min(5, nsc)
            PRE_V = min(3, nsc)
            p1_pending = []

            def defer_kproj(sc):
                xk_ = [None]

                def c_lk(sc=sc):
                    xk_[0] = load_x(kT, sc, nc.sync)
                p1_pending.append(c_lk)
                for f in range(2):
                    def c_kf(sc=sc, f=f):
                        qkproj_emit(sc, xk_[0], wk_sb, bk_sb, ktr, f,
                                    on_act=True)
                    p1_pending.append(c_kf)

            def defer_vproj(sc):
                xv_ = [None]

                def c_lv(sc=sc):
                    xv_[0] = load_x(vT, sc, nc.gpsimd)
                p1_pending.append(c_lv)
                for sub in range(QT // 128):
                    def c_vs(sc=sc, sub=sub):
                        vproj_emit(sc, xv_[0], sub)
                    p1_pending.append(c_vs)

            for sc in range(PRE_K):
                xk = load_x(kT, sc, nc.sync, nc.scalar,
                            nc.gpsimd if sc < 2 else None)
                if sc == min(2, PRE_K - 1):
                    load_late_weights()
                for f in range(2):
                    qkproj_emit(sc, xk, wk_sb, bk_sb, ktr, f, on_act=True)
            for sc in range(PRE_V):
                xv = load_x(vT, sc, nc.gpsimd, nc.scalar)
                for sub in range(QT // 128):
                    vproj_emit(sc, xv, sub)
            for sc in range(min(PRE_K, PRE_V), nsc):
                if sc >= PRE_K:
                    defer_kproj(sc)
                if sc >= PRE_V:
                    defer_vproj(sc)
            # first q chunk up front; the rest interleaves into pair 0.
            xq = load_x(qT, 0, nc.sync)
            for f in range(2):
                qkproj_emit(0, xq, wq_sb, bq_sb, qtr, f, on_act=True)

            # ================= P2: attention =================
            # deferred-emission queue: small closures (qproj chunks,
            # epilogue chunks, Wo chunks) drained between slots.
            pending = []

            def drain(n=1):
                for _ in range(n):
                    if pending:
                        pending.pop(0)()

            def make_epi_chunks(pair, qt, u_sbs, last=False):
                """normalize U' -> O^T; pair0 -> ot0 resident, pair1 ->
                ot_acc tile; pair1 also appends the Wo chunks for qt.

                U' = [O_unnorm; denom] is [65, QT] with q on the free dim,
                which is already the layout ot0/Wo want -- no transposes.
                The per-q normalize: reciprocal of the denom row [1, QT],
                partition-broadcast it (DMA for the 'dma' path, K=1 PE
                matmul otherwise), then one fused DVE multiply straight
                into the resident O^T tile."""
                chunks = []
                if pair == 1:
                    ot_acc = otp.tile([128, QT], dt, tag="ot_acc",
                                      name="ot_acc")
                else:
                    ot_acc = None
                recs = [None, None]
                rbs = [None, None]
                dens = [None, None]
                # plain vector.reciprocal is ~8 cyc/elem (3.3us for
                # [1,512]); approx_fast is ~5x cheaper at 18 bits but NaNs
                # on nonzero base partitions, so stage the denom row to
                # partition 0 on the idle GpSimd engine first. The staging
                # copy gets its own drain chunk TWO blocks ahead of the
                # DVE approx -- a same-chunk pairing left the approx at the
                # DVE FIFO head waiting ~1.6us on GpSimd, stalling the exps
                # queued behind it. GpSimd hops hurt on the final tile's
                # serial tail, so stage on DVE there instead.
                stage = nc.vector if last else nc.gpsimd
                for h in range(2):
                    def c_stage(h=h):
                        den0 = epi.tile([1, QT], F32, tag="den0", bufs=2)
                        stage.tensor_copy(
                            den0[:], u_sbs[h][DEPTH:DEPTH + 1, :])
                        dens[h] = den0
                    chunks.append(c_stage)
                for h in range(2):
                    def c_r(h=h):
                        recf = epi.tile([1, QT], F32, tag="recf", bufs=2)
                        rec = epi.tile([1, QT], dt, tag="rec", bufs=2)
                        nc.vector.reciprocal_approx_fast(
                            recf[:], dens[h][:])
                        with nc.allow_low_precision(reason="bf16 recip row"):
                            stage.tensor_copy(rec[:], recf[:])
                        recs[h] = rec
                    chunks.append(c_r)
                for h in range(2):
                    def c_bm(h=h, qt=qt):
                        # recip rec is 2 drain-blocks old -> PE never waits;
                        # the DVE mul waits ~1 block for PE to reach the
                        # bcast, absorbed by DVE slack (same shape as the
                        # old transpose closures). One closure so the ring
                        # slot's reader lands immediately after its writer.
                        bc = ps_tile([128, QT], "bc")
                        nc.tensor.matmul(
                            bc[0:DEPTH, :], ones_row[0:1, 0:DEPTH],
                            recs[h][:], start=True, stop=True)
                        if pair == 0:
                            dst = ot0[ds(DEPTH * h, DEPTH), ts(qt, QT)]
                        else:
                            dst = ot_acc[ds(DEPTH * h, DEPTH), :]
                        nc.vector.tensor_tensor(
                            dst, u_sbs[h][0:DEPTH, :],
                            bc[0:DEPTH, :], ALU.mult)
                    chunks.append(c_bm)
                if pair == 1:
                    # one (qs, f) per chunk so a drain-block carries at most
                    # one ysb evacuation -- two per block pushed ACT past
                    # its per-block exp budget and stalled the PV stream.
                    # On the final tile the whole list flushes serially, so
                    # spread the evacuations and Y writes across engines.
                    for qs in range(QT // 128):
                        for f in range(2):
                            def c_wo(qs=qs, f=f, ot_acc=ot_acc, qt=qt):
                                yp = ps_tile([128, 512], "yp")
                                nc.tensor.matmul(
                                    yp[:],
                                    ot0[:, ds(qt * QT + qs * 128, 128)],
                                    wo_sb[0][:, ts(f, 512)],
                                    start=True, stop=False)
                                nc.tensor.matmul(
                                    yp[:], ot_acc[:, ts(qs, 128)],
                                    wo_sb[1][:, ts(f, 512)],
                                    start=False, stop=True)
                                ysb = ypool.tile([128, 512], F32, tag="ysb",
                                                 bufs=4)
                                nc.scalar.copy(ysb[:], yp[:])
                                nc.gpsimd.dma_start(
                                    Y[ds(qt * QT + qs * 128, 128),
                                      ts(f, 512)], ysb[:])
                            chunks.append(c_wo)
                return chunks

            def make_qproj_chunks(sc):
                chunks = []
                xq = [None]

                def c_load(sc=sc):
                    xq[0] = load_x(qT, sc, nc.sync)
                chunks.append(c_load)
                for f in range(2):
                    def c_proj(sc=sc, f=f):
                        qkproj_emit(sc, xq[0], wq_sb, bq_sb, qtr, f)
                    chunks.append(c_proj)
                return chunks

            sch_b = SCH_B0 + boff

            tiles_l = [(pair, qt) for pair in range(2)
                       for qt in range(nqt)]
            lg_tiles = {}

            def emit_lg(tidx, i):
                pair_, qt_ = tiles_l[tidx]
                ktp_, qtp_ = ktr[pair_], qtr[pair_]
                lg = ps_logit.tile([128, 2 * QT], F32, tag="lg",
                                   bufs=3, name="lg")
                for h in range(2):
                    nc.tensor.matmul(
                        lg[:, ts(h, QT)],
                        ktp_[ds(DEPTH * h, DEPTH), ts(i, KC)],
                        qtp_[ds(DEPTH * h, DEPTH), ts(qt_, QT)],
                        start=True, stop=True,
                        tile_position=(DEPTH * h, 0))
                lg_tiles[(tidx, i)] = lg

            LOOK = 4
            pts_g = {}

            def emit_exp(tidx2, j2):
                lg = lg_tiles.pop((tidx2, j2))
                use_dve = (((j2 + 1) * dve_num) // dve_den) != (
                    (j2 * dve_num) // dve_den)
                if use_dve:
                    pt_i = ppool.tile([128, 2 * QT], I16,
                                      tag="ptD", bufs=4, name="ptD")
                    nc.vector.tensor_scalar(
                        pt_i[:], lg[:], SCH_A, sch_b,
                        op0=ALU.mult, op1=ALU.add)
                    pts_g[(tidx2, j2)] = pt_i[:].bitcast(dt)
                else:
                    pt_b = ppool.tile([128, 2 * QT], dt,
                                      tag="ptA", bufs=4, name="ptA")
                    nc.scalar.activation(
                        pt_b[:], lg[:], AFT.Exp, scale=SCALE)
                    pts_g[(tidx2, j2)] = pt_b[:]

            for tidx, (pair, qt) in enumerate(tiles_l):
                vpp = vpr[pair]
                if pair == 0 and qt + 1 < nsc:
                    pending.extend(make_qproj_chunks(qt + 1))
                pv_ps = [ps_pv.tile([DEPTH + 1, QT], F32,
                                    tag=f"pv{h}", bufs=1, name=f"pv{h}")
                         for h in range(2)]
                if tidx == 0:
                    for i in range(min(LOOK, nkc)):
                        emit_lg(0, i)
                    emit_exp(0, 0)
                    emit_exp(0, 1)
                # software-pipelined loop over BLOCKS of 2 k-chunks.
                # Per block: pv(j) x4 (pts produced a block ago), lg(j+4)
                # x2, exp(j+2) x2 for the NEXT block. Exps are emitted a
                # block early so they sit AHEAD of the drained closures'
                # ACT/DVE ops in the engine queues -- otherwise those ops
                # delay the exp and the next block's first PV stalls. The
                # pv-before-lg order gives every matmul exactly one
                # pending weight-load, and both the lg and exp lookahead
                # run ACROSS tile boundaries so the PE never drains at
                # tile turns.
                if True:
                    for b0 in range(0, nkc, 2):
                        blk = [j for j in (b0, b0 + 1) if j < nkc]
                        for j in blk:
                            pt = pts_g.pop((tidx, j))
                            for h in range(2):
                                nc.tensor.matmul(
                                    pv_ps[h][:],
                                    vpp[:, j,
                                        ds(h * (DEPTH + 1), DEPTH + 1)],
                                    pt[:, ts(h, QT)],
                                    start=(j == 0), stop=(j == nkc - 1))
                        for j in blk:
                            ahead = j + LOOK
                            if ahead < nkc:
                                emit_lg(tidx, ahead)
                            elif tidx + 1 < len(tiles_l):
                                emit_lg(tidx + 1, ahead - nkc)
                        for j in blk:
                            nxt = j + 2
                            if nxt < nkc:
                                emit_exp(tidx, nxt)
                            elif tidx + 1 < len(tiles_l):
                                emit_exp(tidx + 1, nxt - nkc)
                        for _ in range(3):
                            if p1_pending:
                                p1_pending.pop(0)()
                        drain(1)

                # spill U' out of PSUM (frees pv banks), defer the rest;
                # one spill on ACT, one on DVE so neither engine eats a
                # 1.4us boundary spike that would delay its next exp.
                u_sbs = []
                for h in range(2):
                    u_sb = epi.tile([DEPTH + 1, QT], F32, tag="u_sb",
                                    bufs=6)
                    if h == 0:
                        nc.scalar.copy(u_sb[:], pv_ps[h][:])
                    else:
                        nc.vector.tensor_copy(u_sb[:], pv_ps[h][:])
                    u_sbs.append(u_sb)
                pending.extend(make_epi_chunks(
                    pair, qt, u_sbs, last=(tidx == len(tiles_l) - 1)))
            drain(len(pending))
    nc.compile()
    return nc


_NC_CACHE = {}


def _get_program(key_args):
    if key_args not in _NC_CACHE:
        _NC_CACHE[key_args] = build_program(*key_args)
    return _NC_CACHE[key_args]


def make_in_maps(inputs, seq=S):
    """Host-side sharding: per-core input dicts."""
    try:
        import ml_dtypes
        bf16 = ml_dtypes.bfloat16
    except ImportError:
        bf16 = None

    def cast(x):
        return x.astype(bf16)

    q = np.asarray(inputs["q"], np.float32)
    k = np.asarray(inputs["k"], np.float32)
    v = np.asarray(inputs["v"], np.float32)
    Wq = np.asarray(inputs["Wq"], np.float32)
    Wk = np.asarray(inputs["Wk"], np.float32)
    Wv = np.asarray(inputs["Wv"], np.float32)
    Wo = np.asarray(inputs["Wo"], np.float32)
    bq = np.asarray(inputs["bq"], np.float32)
    bk = np.asarray(inputs["bk"], np.float32)

    qTb = [np.ascontiguousarray(q[b].T) for b in range(B)]
    kTb = [np.ascontiguousarray(k[b].T) for b in range(B)]
    vTb = [np.ascontiguousarray(v[b].T) for b in range(B)]

    in_maps = []
    for c in range(8):
        b, g = c // G, c % G
        cols = slice(g * DG, (g + 1) * DG)
        in_maps.append({
            "qT": cast(qTb[b]), "kT": cast(kTb[b]), "vT": cast(vTb[b]),
            "Wq": cast(np.ascontiguousarray(Wq[:, cols])),
            "Wk": cast(np.ascontiguousarray(Wk[:, cols])),
            "Wv": cast(np.ascontiguousarray(Wv[:, cols])),
            "Wo": cast(np.ascontiguousarray(Wo[cols, :])),
            "bq": np.ascontiguousarray(bq[cols].reshape(DG, 1)),
            "bk": np.ascontiguousarray(bk[cols].reshape(DG, 1)),
        })
    return in_maps


LAST_RESULT = None


def kernel(**inputs):
    global LAST_RESULT
    dve_num = int(os.environ.get("MHA_DVE_NUM", "1"))
    dve_den = int(os.environ.get("MHA_DVE_DEN", "2"))
    boff = float(os.environ.get("MHA_BOFF", "-7.4"))
    nc = _get_program((S, dve_num, dve_den, boff))
    in_maps = make_in_maps(inputs, S)
    res = run_bass_kernel_spmd(nc, in_maps, list(range(8)))
    LAST_RESULT = res
    bo = np.asarray(inputs["bo"], np.float64)
    bv = np.asarray(inputs["bv"], np.float64)
    Wo = np.asarray(inputs["Wo"], np.float64)
    out = np.zeros((B, S, D), np.float32)
    for c in range(8):
        b = c // G
        out[b] += res.results[c]["Y"]
    # the device kernel drops the V bias; its exact effect on Y is the
    # constant row bv @ Wo (plus the output bias bo), added here.
    out += (bo + bv @ Wo)[None, None, :].astype(np.float32)
    return out


if __name__ == "__main__":
    # smoke build
    nc = build_program(1024)
    print("built ok")



# revision 49
# speedup vs baseline: 1.0049x; 1.0049x over previous
"""Trainium2 Bass kernel for nn_MultiHeadAttention_67731634258682.

MHA: B=2, S=8192, D=1024, H=16 heads (depth 64).
Sharding over 8 cores: core c -> (batch b = c//4, head-group g = c%4).
Each core computes its 4 heads end-to-end plus a row-parallel partial of
the output projection; the host sums the 4 partials per batch.

Design (measured 1.88 ms; the earlier session's kernel was 2.06 ms and
the staged nn baseline 3.09 ms on this box):
  - Everything SBUF-resident: K^T/Q^T (2 pairs x [128, S]), V' (2 pairs x
    [128, nkc, 130] with ones columns), pair-0 O^T [128, S]. Projections
    evacuate straight into the resident tiles (no DRAM scratch round-trip).
  - Exp split 1:1 across engines by k-chunk parity: ScalarE exact Exp
    (softmax scale folded into the activation) / DVE Schraudolph
    bit-trick: bf16 bits = int16(round(A*logit + B)), one fused
    tensor_scalar per slot. Rel-err measured 1.42e-2 (gate 2e-2).
  - k-loop processes BLOCKS of 2 k-chunks with a 3-deep logits PSUM ring
    (3 x 2 banks, shared by all small PE outputs) + 2 pinned PV banks.
    Per block: pv(j) x4 (P tiles made a block ago), lg(j+4) x2 row-tiled
    pairs (tile_position (0,0)/(64,0) -> the two heads' QK^T matmuls
    stream CONCURRENTLY), exp(j+2) x2 emitted a block early so they sit
    ahead of drained-closure ops in the ACT/DVE queues; lg and exp
    lookahead both run across tile boundaries so the PE never drains at
    a tile turn. pv-before-lg leaves every matmul one pending LDWEIGHTS.
  - Epilogue per tile (no PE transposes): U'=[O_unnorm^T; denom] spills
    to SBUF (one head on ACT, one on DVE), then deferred closures:
    denom row staged to partition 0 on GpSimd, reciprocal_approx_fast on
    DVE (plain reciprocal is ~8 cyc/elem; approx NaNs on nonzero base
    partitions, hence the staging), bf16 cast on GpSimd, K=1 ones-row
    matmul broadcasts the recip row across partitions, one fused DVE
    tensor_tensor multiply writes the resident O^T. Wo chunks are one
    (qs, f) each so a drain block carries at most one ysb evacuation.
  - V-projection carries no bias: sum p*(v+bv) = sum p*v + bv*denom,
    i.e. +bv after normalization, i.e. +bv@Wo on Y -- a constant row
    added on the host (exact, in float64).
  - Only the first few s-chunks of the K/V projections run up front
    (split across two DMA queues; ~145 GB/s per queue on this strided
    pattern); the rest drain as deadline-ordered closures inside the
    first attention tile. Trailing weights (Wv/Wq/Wo/bq) issue after
    the first k-chunk loads so they never delay the first matmul.
"""

import os
import sys
import numpy as np

for _p in ("/opt/trn_rl_repo", "/root/.axon_site/_ro/trn_rl_repo"):
    if os.path.isdir(_p) and _p not in sys.path:
        sys.path.append(_p)

import concourse.bass as bass
import concourse.mybir as mybir
from concourse import bacc, tile
from concourse.bass import ts, ds
from concourse.bass_utils import run_bass_kernel_spmd

F32 = mybir.dt.float32
BF16 = mybir.dt.bfloat16
I16 = mybir.dt.int16

B, S, D = 2, 8192, 1024
H = 16
DEPTH = 64          # head dim
G = 4               # head groups (one per core within a batch)
HPG = 4             # heads per group
DG = HPG * DEPTH    # 256 features per group
QT = 512            # q tile
KC = 128            # k chunk (matmul contraction tile)
NDC = D // 128      # 8 contraction chunks for projections

AFT = mybir.ActivationFunctionType
ALU = mybir.AluOpType

SCALE = 0.125                                  # 1/sqrt(64)
SCH_A = SCALE * np.log2(np.e) * 128.0          # schraudolph multiplier
SCH_B0 = 127.0 * 128.0                         # exponent bias in bf16 bits


def build_program(seq=S, dve_num=1, dve_den=3, boff=-7.4):
    """Build the per-core Bass program. Returns the compiled Bacc object."""
    assert seq % QT == 0
    nqt = seq // QT
    nkc = seq // KC
    nsc = seq // QT
    dt = BF16

    nc = bacc.Bacc("TRN2", target_bir_lowering=False, debug=False,
                   enable_asserts=False, num_devices=8)

    # ---- external I/O ----
    qT = nc.dram_tensor("qT", [D, seq], dt, kind="ExternalInput").ap()
    kT = nc.dram_tensor("kT", [D, seq], dt, kind="ExternalInput").ap()
    vT = nc.dram_tensor("vT", [D, seq], dt, kind="ExternalInput").ap()
    Wq = nc.dram_tensor("Wq", [D, DG], dt, kind="ExternalInput").ap()
    Wk = nc.dram_tensor("Wk", [D, DG], dt, kind="ExternalInput").ap()
    Wv = nc.dram_tensor("Wv", [D, DG], dt, kind="ExternalInput").ap()
    Wo = nc.dram_tensor("Wo", [DG, D], dt, kind="ExternalInput").ap()
    bq = nc.dram_tensor("bq", [DG, 1], F32, kind="ExternalInput").ap()
    bk = nc.dram_tensor("bk", [DG, 1], F32, kind="ExternalInput").ap()
    Y = nc.dram_tensor("Y", [seq, D], F32, kind="ExternalOutput").ap()

    with tile.TileContext(nc) as tc:
        from contextlib import ExitStack
        ctx = ExitStack()
        with ctx:
            const = ctx.enter_context(tc.tile_pool(name="const", bufs=1))
            res = ctx.enter_context(tc.tile_pool(name="res", bufs=1))
            xin = ctx.enter_context(tc.tile_pool(name="xin", bufs=3))
            ppool = ctx.enter_context(tc.tile_pool(name="ppool", bufs=3))
            epi = ctx.enter_context(tc.tile_pool(name="epi", bufs=4))
            otp = ctx.enter_context(tc.tile_pool(name="otp", bufs=3))
            ypool = ctx.enter_context(tc.tile_pool(name="ypool", bufs=3))
            # One shared PSUM ring: 3 slots of 2 banks each (tag "lg") serve
            # the logits tiles AND all small PE outputs (transposes, Wo
            # accumulator, projection accumulator); pv pins the last 2 banks.
            ps_logit = ctx.enter_context(
                tc.tile_pool(name="ps_logit", bufs=3, space="PSUM"))
            ps_pv = ctx.enter_context(
                tc.tile_pool(name="ps_pv", bufs=1, space="PSUM"))

            def ps_tile(shape, name):
                return ps_logit.tile(shape, F32, tag="lg", bufs=3, name=name)

            # ---- constants ----
            ones_f32 = const.tile([128, 128], F32, tag="ones_f32")
            nc.any.memset(ones_f32[:], 1.0)
            ones_row = const.tile([1, 128], dt, tag="ones_row")
            nc.vector.tensor_copy(ones_row[:], ones_f32[0:1, :])

            # Startup DMA order matters: the first k-proj matmul needs wk +
            # bk + the first kT chunk. Spread issues across engine queues
            # (issue cost ~0.6us each on one queue) and front-load the
            # K-path; wq/wo can trail (first used much later).
            wq_sb = [const.tile([128, DG], dt, tag=f"wq{dc}", name=f"wq{dc}")
                     for dc in range(NDC)]
            wk_sb = [const.tile([128, DG], dt, tag=f"wk{dc}", name=f"wk{dc}")
                     for dc in range(NDC)]
            wv_sb = [const.tile([128, DG], dt, tag=f"wv{dc}", name=f"wv{dc}")
                     for dc in range(NDC)]
            qs_engines = [nc.scalar, nc.gpsimd]
            bk_sb = [const.tile([128, 1], F32, tag=f"bk{i}", name=f"bk{i}")
                     for i in range(2)]
            bq_sb = [const.tile([128, 1], F32, tag=f"bq{i}", name=f"bq{i}")
                     for i in range(2)]
            for dc in range(NDC):
                qs_engines[dc % 2].dma_start(wk_sb[dc][:], Wk[ts(dc, 128), :])
            for i in range(2):
                qs_engines[i].dma_start(bk_sb[i][:], bk[ts(i, 128), :])
            wo_sb = [const.tile([128, D], dt, tag=f"wo{i}", name=f"wo{i}")
                     for i in range(2)]

            def load_late_weights():
                # issued after the PRE_K x loads so the scalar DMA queue
                # serves the first k-chunks before these trailing weights.
                for dc in range(NDC):
                    qs_engines[dc % 2].dma_start(
                        wv_sb[dc][:], Wv[ts(dc, 128), :])
                for dc in range(NDC):
                    qs_engines[dc % 2].dma_start(
                        wq_sb[dc][:], Wq[ts(dc, 128), :])
                for i in range(2):
                    qs_engines[i].dma_start(bq_sb[i][:], bq[ts(i, 128), :])
                for i in range(2):
                    qs_engines[i].dma_start(wo_sb[i][:], Wo[ts(i, 128), :])

            # ---- resident tensors (persist for the whole kernel) ----
            ktr = [res.tile([128, seq], dt, tag=f"ktr{p}", name=f"ktr{p}")
                   for p in range(2)]
            qtr = [res.tile([128, seq], dt, tag=f"qtr{p}", name=f"qtr{p}")
                   for p in range(2)]
            VP_W = 2 * (DEPTH + 1)  # per-pair per-chunk: 2 heads x [V_h|1]
            vpr = [res.tile([128, nkc, VP_W], dt, tag=f"vpr{p}",
                            name=f"vpr{p}") for p in range(2)]
            ot0 = res.tile([128, seq], dt, tag="ot0", name="ot0")
            # ones columns of V' are constant: write once.
            for p in range(2):
                vh = vpr[p][:].rearrange("p k (h x) -> p k h x", x=DEPTH + 1)
                nc.vector.memset(vh[:, :, :, DEPTH:DEPTH + 1], 1.0)

            # ---- projection emitters ----
            def load_x(src, sc, eng, eng2=None):
                # eng2 splits the 1MB strided load across two DMA queues
                # (one engine sustains only ~145GB/s on this pattern) --
                # used for the up-front P1 loads that gate the first tile.
                xt = xin.tile([128, NDC, QT], dt, tag="xt", name="xt")
                rr = src.rearrange("(c p) s -> p c s", p=128)
                if eng2 is None:
                    eng.dma_start(xt[:], rr[:, :, ts(sc, QT)])
                else:
                    hc = NDC // 2
                    eng.dma_start(xt[:, 0:hc, :],
                                  rr[:, 0:hc, ts(sc, QT)])
                    eng2.dma_start(xt[:, hc:NDC, :],
                                   rr[:, hc:NDC, ts(sc, QT)])
                return xt

            def qkproj_emit(sc, xt, w_sb, b_sb, dst, f, on_act=False):
                ps = ps_tile([128, QT], "ps")
                for dc in range(NDC):
                    nc.tensor.matmul(
                        ps[:], w_sb[dc][:, ts(f, 128)], xt[:, dc, :],
                        start=(dc == 0), stop=(dc == NDC - 1))
                if on_act:
                    # P1 runs before attention: ScalarE is idle there, so
                    # evacuate on it to keep DVE free.
                    nc.scalar.activation(
                        dst[f][:, ts(sc, QT)], ps[:], AFT.Identity,
                        bias=b_sb[f][:])
                else:
                    nc.vector.tensor_scalar_add(
                        dst[f][:, ts(sc, QT)], ps[:], b_sb[f][:])

            def vproj_emit(sc, xt, sub):
                # no bias ride-along: out = sum p*(v+bv) differs from
                # sum p*v by bv*denom, i.e. +bv after normalization, i.e.
                # +bv@Wo on Y -- a constant row the host adds exactly.
                ps = ps_tile([128, DG], "ps")
                for dc in range(NDC):
                    nc.tensor.matmul(
                        ps[:], xt[:, dc, ts(sub, 128)], wv_sb[dc][:],
                        start=(dc == 0), stop=(dc == NDC - 1))
                kc_i = sc * (QT // 128) + sub
                for p in range(2):
                    src = ps[:, ds(p * 2 * DEPTH, 2 * DEPTH)].rearrange(
                        "p (h x) -> p h x", x=DEPTH)
                    dstv = vpr[p][:, kc_i, :].rearrange(
                        "p (h x) -> p h x", x=DEPTH + 1)
                    nc.scalar.copy(dstv[:, :, 0:DEPTH], src)

            # ================= P1: K + V projections =================
            # Only the first few s-chunks are projected up front (the ones
            # the first attention tile touches before its deferred chunks
            # can land); the rest are deadline-ordered closures drained
            # into tile (0,0)'s k-loop, filling its coupling idle.
            PRE_K = min(5, nsc)
            PRE_V = min(3, nsc)
            p1_pending = []

            def defer_kproj(sc):
                xk_ = [None]

                def c_lk(sc=sc):
                    xk_[0] = load_x(kT, sc, nc.sync)
                p1_pending.append(c_lk)
                for f in range(2):
                    def c_kf(sc=sc, f=f):
                        qkproj_emit(sc, xk_[0], wk_sb, bk_sb, ktr, f,
                                    on_act=True)
                    p1_pending.append(c_kf)

            def defer_vproj(sc):
                xv_ = [None]

                def c_lv(sc=sc):
                    xv_[0] = load_x(vT, sc, nc.gpsimd)
                p1_pending.append(c_lv)
                for sub in range(QT // 128):
                    def c_vs(sc=sc, sub=sub):
                        vproj_emit(sc, xv_[0], sub)
                    p1_pending.append(c_vs)

            for sc in range(PRE_K):
                xk = load_x(kT, sc, nc.sync, nc.scalar)
                if sc == min(2, PRE_K - 1):
                    load_late_weights()
                for f in range(2):
                    qkproj_emit(sc, xk, wk_sb, bk_sb, ktr, f, on_act=True)
            for sc in range(PRE_V):
                xv = load_x(vT, sc, nc.gpsimd, nc.scalar)
                for sub in range(QT // 128):
                    vproj_emit(sc, xv, sub)
            for sc in range(min(PRE_K, PRE_V), nsc):
                if sc >= PRE_K:
                    defer_kproj(sc)
                if sc >= PRE_V:
                    defer_vproj(sc)
            # first q chunk up front; the rest interleaves into pair 0.
            xq = load_x(qT, 0, nc.sync)
            for f in range(2):
                qkproj_emit(0, xq, wq_sb, bq_sb, qtr, f, on_act=True)

            # ================= P2: attention =================
            # deferred-emission queue: small closures (qproj chunks,
            # epilogue chunks, Wo chunks) drained between slots.
            pending = []

            def drain(n=1):
                for _ in range(n):
                    if pending:
                        pending.pop(0)()

            def make_epi_chunks(pair, qt, u_sbs, last=False):
                """normalize U' -> O^T; pair0 -> ot0 resident, pair1 ->
                ot_acc tile; pair1 also appends the Wo chunks for qt.

                U' = [O_unnorm; denom] is [65, QT] with q on the free dim,
                which is already the layout ot0/Wo want -- no transposes.
                The per-q normalize: reciprocal of the denom row [1, QT],
                partition-broadcast it (DMA for the 'dma' path, K=1 PE
                matmul otherwise), then one fused DVE multiply straight
                into the resident O^T tile."""
                chunks = []
                if pair == 1:
                    ot_acc = otp.tile([128, QT], dt, tag="ot_acc",
                                      name="ot_acc")
                else:
                    ot_acc = None
                recs = [None, None]
                rbs = [None, None]
                dens = [None, None]
                # plain vector.reciprocal is ~8 cyc/elem (3.3us for
                # [1,512]); approx_fast is ~5x cheaper at 18 bits but NaNs
                # on nonzero base partitions, so stage the denom row to
                # partition 0 on the idle GpSimd engine first. The staging
                # copy gets its own drain chunk TWO blocks ahead of the
                # DVE approx -- a same-chunk pairing left the approx at the
                # DVE FIFO head waiting ~1.6us on GpSimd, stalling the exps
                # queued behind it. GpSimd hops hurt on the final tile's
                # serial tail, so stage on DVE there instead.
                stage = nc.vector if last else nc.gpsimd
                for h in range(2):
                    def c_stage(h=h):
                        den0 = epi.tile([1, QT], F32, tag="den0", bufs=2)
                        stage.tensor_copy(
                            den0[:], u_sbs[h][DEPTH:DEPTH + 1, :])
                        dens[h] = den0
                    chunks.append(c_stage)
                for h in range(2):
                    def c_r(h=h):
                        recf = epi.tile([1, QT], F32, tag="recf", bufs=2)
                        rec = epi.tile([1, QT], dt, tag="rec", bufs=2)
                        nc.vector.reciprocal_approx_fast(
                            recf[:], dens[h][:])
                        with nc.allow_low_precision(reason="bf16 recip row"):
                            stage.tensor_copy(rec[:], recf[:])
                        recs[h] = rec
                    chunks.append(c_r)
                for h in range(2):
                    def c_bm(h=h, qt=qt):
                        # recip rec is 2 drain-blocks old -> PE never waits;
                        # the DVE mul waits ~1 block for PE to reach the
                        # bcast, absorbed by DVE slack (same shape as the
                        # old transpose closures). One closure so the ring
                        # slot's reader lands immediately after its writer.
                        bc = ps_tile([128, QT], "bc")
                        nc.tensor.matmul(
                            bc[0:DEPTH, :], ones_row[0:1, 0:DEPTH],
                            recs[h][:], start=True, stop=True)
                        if pair == 0:
                            dst = ot0[ds(DEPTH * h, DEPTH), ts(qt, QT)]
                        else:
                            dst = ot_acc[ds(DEPTH * h, DEPTH), :]
                        nc.vector.tensor_tensor(
                            dst, u_sbs[h][0:DEPTH, :],
                            bc[0:DEPTH, :], ALU.mult)
                    chunks.append(c_bm)
                if pair == 1:
                    # one (qs, f) per chunk so a drain-block carries at most
                    # one ysb evacuation -- two per block pushed ACT past
                    # its per-block exp budget and stalled the PV stream.
                    # On the final tile the whole list flushes serially, so
                    # spread the evacuations and Y writes across engines.
                    for qs in range(QT // 128):
                        for f in range(2):
                            def c_wo(qs=qs, f=f, ot_acc=ot_acc, qt=qt):
                                yp = ps_tile([128, 512], "yp")
                                nc.tensor.matmul(
                                    yp[:],
                                    ot0[:, ds(qt * QT + qs * 128, 128)],
                                    wo_sb[0][:, ts(f, 512)],
                                    start=True, stop=False)
                                nc.tensor.matmul(
                                    yp[:], ot_acc[:, ts(qs, 128)],
                                    wo_sb[1][:, ts(f, 512)],
                                    start=False, stop=True)
                                ysb = ypool.tile([128, 512], F32, tag="ysb",
                                                 bufs=4)
                                nc.scalar.copy(ysb[:], yp[:])
                                nc.gpsimd.dma_start(
                                    Y[ds(qt * QT + qs * 128, 128),
                                      ts(f, 512)], ysb[:])
                            chunks.append(c_wo)
                return chunks

            def make_qproj_chunks(sc):
                chunks = []
                xq = [None]

                def c_load(sc=sc):
                    xq[0] = load_x(qT, sc, nc.sync)
                chunks.append(c_load)
                for f in range(2):
                    def c_proj(sc=sc, f=f):
                        qkproj_emit(sc, xq[0], wq_sb, bq_sb, qtr, f)
                    chunks.append(c_proj)
                return chunks

            sch_b = SCH_B0 + boff

            tiles_l = [(pair, qt) for pair in range(2)
                       for qt in range(nqt)]
            lg_tiles = {}

            def emit_lg(tidx, i):
                pair_, qt_ = tiles_l[tidx]
                ktp_, qtp_ = ktr[pair_], qtr[pair_]
                lg = ps_logit.tile([128, 2 * QT], F32, tag="lg",
                                   bufs=3, name="lg")
                for h in range(2):
                    nc.tensor.matmul(
                        lg[:, ts(h, QT)],
                        ktp_[ds(DEPTH * h, DEPTH), ts(i, KC)],
                        qtp_[ds(DEPTH * h, DEPTH), ts(qt_, QT)],
                        start=True, stop=True,
                        tile_position=(DEPTH * h, 0))
                lg_tiles[(tidx, i)] = lg

            LOOK = 4
            pts_g = {}

            def emit_exp(tidx2, j2):
                lg = lg_tiles.pop((tidx2, j2))
                use_dve = (((j2 + 1) * dve_num) // dve_den) != (
                    (j2 * dve_num) // dve_den)
                if use_dve:
                    pt_i = ppool.tile([128, 2 * QT], I16,
                                      tag="ptD", bufs=4, name="ptD")
                    nc.vector.tensor_scalar(
                        pt_i[:], lg[:], SCH_A, sch_b,
                        op0=ALU.mult, op1=ALU.add)
                    pts_g[(tidx2, j2)] = pt_i[:].bitcast(dt)
                else:
                    pt_b = ppool.tile([128, 2 * QT], dt,
                                      tag="ptA", bufs=4, name="ptA")
                    nc.scalar.activation(
                        pt_b[:], lg[:], AFT.Exp, scale=SCALE)
                    pts_g[(tidx2, j2)] = pt_b[:]

            for tidx, (pair, qt) in enumerate(tiles_l):
                vpp = vpr[pair]
                if pair == 0 and qt + 1 < nsc:
                    pending.extend(make_qproj_chunks(qt + 1))
                pv_ps = [ps_pv.tile([DEPTH + 1, QT], F32,
                                    tag=f"pv{h}", bufs=1, name=f"pv{h}")
                         for h in range(2)]
                if tidx == 0:
                    for i in range(min(LOOK, nkc)):
                        emit_lg(0, i)
                    emit_exp(0, 0)
                    emit_exp(0, 1)
                # software-pipelined loop over BLOCKS of 2 k-chunks.
                # Per block: pv(j) x4 (pts produced a block ago), lg(j+4)
                # x2, exp(j+2) x2 for the NEXT block. Exps are emitted a
                # block early so they sit AHEAD of the drained closures'
                # ACT/DVE ops in the engine queues -- otherwise those ops
                # delay the exp and the next block's first PV stalls. The
                # pv-before-lg order gives every matmul exactly one
                # pending weight-load, and both the lg and exp lookahead
                # run ACROSS tile boundaries so the PE never drains at
                # tile turns.
                if True:
                    for b0 in range(0, nkc, 2):
                        blk = [j for j in (b0, b0 + 1) if j < nkc]
                        for j in blk:
                            pt = pts_g.pop((tidx, j))
                            for h in range(2):
                                nc.tensor.matmul(
                                    pv_ps[h][:],
                                    vpp[:, j,
                                        ds(h * (DEPTH + 1), DEPTH + 1)],
                                    pt[:, ts(h, QT)],
                                    start=(j == 0), stop=(j == nkc - 1))
                        for j in blk:
                            ahead = j + LOOK
                            if ahead < nkc:
                                emit_lg(tidx, ahead)
                            elif tidx + 1 < len(tiles_l):
                                emit_lg(tidx + 1, ahead - nkc)
                        for j in blk:
                            nxt = j + 2
                            if nxt < nkc:
                                emit_exp(tidx, nxt)
                            elif tidx + 1 < len(tiles_l):
                                emit_exp(tidx + 1, nxt - nkc)
                        for _ in range(3):
                            if p1_pending:
                                p1_pending.pop(0)()
                        drain(1)

                # spill U' out of PSUM (frees pv banks), defer the rest;
                # one spill on ACT, one on DVE so neither engine eats a
                # 1.4us boundary spike that would delay its next exp.
                u_sbs = []
                for h in range(2):
                    u_sb = epi.tile([DEPTH + 1, QT], F32, tag="u_sb",
                                    bufs=6)
                    if h == 0:
                        nc.scalar.copy(u_sb[:], pv_ps[h][:])
                    else:
                        nc.vector.tensor_copy(u_sb[:], pv_ps[h][:])
                    u_sbs.append(u_sb)
                pending.extend(make_epi_chunks(
                    pair, qt, u_sbs, last=(tidx == len(tiles_l) - 1)))
            drain(len(pending))
    nc.compile()
    return nc


_NC_CACHE = {}


def _get_program(key_args):
    if key_args not in _NC_CACHE:
        _NC_CACHE[key_args] = build_program(*key_args)
    return _NC_CACHE[key_args]


def make_in_maps(inputs, seq=S):
    """Host-side sharding: per-core input dicts."""
    try:
        import ml_dtypes
        bf16 = ml_dtypes.bfloat16
    except ImportError:
        bf16 = None

    def cast(x):
        return x.astype(bf16)

    q = np.asarray(inputs["q"], np.float32)
    k = np.asarray(inputs["k"], np.float32)
    v = np.asarray(inputs["v"], np.float32)
    Wq = np.asarray(inputs["Wq"], np.float32)
    Wk = np.asarray(inputs["Wk"], np.float32)
    Wv = np.asarray(inputs["Wv"], np.float32)
    Wo = np.asarray(inputs["Wo"], np.float32)
    bq = np.asarray(inputs["bq"], np.float32)
    bk = np.asarray(inputs["bk"], np.float32)

    qTb = [np.ascontiguousarray(q[b].T) for b in range(B)]
    kTb = [np.ascontiguousarray(k[b].T) for b in range(B)]
    vTb = [np.ascontiguousarray(v[b].T) for b in range(B)]

    in_maps = []
    for c in range(8):
        b, g = c // G, c % G
        cols = slice(g * DG, (g + 1) * DG)
        in_maps.append({
            "qT": cast(qTb[b]), "kT": cast(kTb[b]), "vT": cast(vTb[b]),
            "Wq": cast(np.ascontiguousarray(Wq[:, cols])),
            "Wk": cast(np.ascontiguousarray(Wk[:, cols])),
            "Wv": cast(np.ascontiguousarray(Wv[:, cols])),
            "Wo": cast(np.ascontiguousarray(Wo[cols, :])),
            "bq": np.ascontiguousarray(bq[cols].reshape(DG, 1)),
            "bk": np.ascontiguousarray(bk[cols].reshape(DG, 1)),
        })
    return in_maps


LAST_RESULT = None


def kernel(**inputs):
    global LAST_RESULT
    dve_num = int(os.environ.get("MHA_DVE_NUM", "1"))
    dve_den = int(os.environ.get("MHA_DVE_DEN", "2"))
    boff = float(os.environ.get("MHA_BOFF", "-7.4"))
    nc = _get_program((S, dve_num, dve_den, boff))
    in_maps = make_in_maps(inputs, S)
    res = run_bass_kernel_spmd(nc, in_maps, list(range(8)))
    LAST_RESULT = res
    bo = np.asarray(inputs["bo"], np.float64)
    bv = np.asarray(inputs["bv"], np.float64)
    Wo = np.asarray(inputs["Wo"], np.float64)
    out = np.zeros((B, S, D), np.float32)
    for c in range(8):
        b = c // G
        out[b] += res.results[c]["Y"]
    # the device kernel drops the V bias; its exact effect on Y is the
    # constant row bv @ Wo (plus the output bias bo), added here.
    out += (bo + bv @ Wo)[None, None, :].astype(np.float32)
    return out


if __name__ == "__main__":
    # smoke build
    nc = build_program(1024)
    print("built ok")

